# revision 35
# baseline (speedup 1.0000x reference)
"""Causal self-attention (GQA) Trainium2 Bass kernel.

Config (hardcoded): B=2, T=2048, C=4096, n_head=32, n_query_groups=8,
head_size=128, qkv_out=6144. RoPE (NeoX, rotary dim = head_size).

Sharding: tensor-parallel over the 8 KV groups, one group per NeuronCore.
Each core computes its group's QKV projection, RoPE, causal attention for
its 4 query heads, and a partial output projection against its 512 columns
of W_proj. The 8 fp32 partials are summed on the host (the all-reduce is
host-side since full outputs are gathered anyway).

Everything on the PE runs in float32r (fp32 storage, PE rounds to its
fast-fp32 format): ~1.5e-4 matmul rel err vs 2.3e-3 for bf16, at full PE
rate when the moving dim is >=256.

Attention is computed in the S^T = K@Q^T orientation so the softmax
reduction lands on the free axis sum via a ones-column in the PV matmul,
and no P-transposes are needed (only V and the final per-head Y tiles get
PE transposes).
"""

import math
from contextlib import ExitStack

import numpy as np

import concourse.bass as bass
import concourse.tile as tile
from concourse import mybir
from concourse.bass_utils import run_bass_kernel_spmd
from concourse.tile import TileContext
from concourse.vector_clock import ScopedClock

# ---------------------------------------------------------------------------
# Workarounds: walrus codegen on this toolchain rejects >1 sync wait per
# instruction ("Too many sync wait commands"). Split excess waits onto
# extra SP drains (TileContext tail) / same-engine InstNoOp instructions.
# ---------------------------------------------------------------------------

_MAX_WAITS = 1


def _patched_drain_and_barrier(self, tick_clock, wait_clock):
    drain_inst = self.nc.sync.drain()
    wait_clock.add_sem_waits(
        drain_inst.ins, ScopedClock({None: tick_clock.global_clock})
    )
    si = drain_inst.ins.sync_info
    waits = list(si.on_wait or [])
    if len(waits) > _MAX_WAITS:
        si.on_wait = waits[:_MAX_WAITS]
        rest = waits[_MAX_WAITS:]
        for i in range(0, len(rest), _MAX_WAITS):
            extra = self.nc.sync.drain()
            extra.ins.sync_info = type(si)(
                on_wait=rest[i : i + _MAX_WAITS], on_update=[]
            )

    self.nc.all_engine_barrier()
    assert self.sems is not None
    popped = self.nc._tile_sem_poison_stack.pop()
    assert popped is self._sem_poison
    self.nc.clear_and_free_semaphores(list(self.sems.allocated().values()))
    self.nc.all_engine_barrier()


tile.TileContext._drain_and_barrier = _patched_drain_and_barrier

_split_counter = [0]


def _split_multi_waits(nc, max_waits=_MAX_WAITS):
    for fn in nc.m.functions:
        for bb in fn.blocks:
            out = []
            for inst in bb.instructions:
                si = inst.sync_info
                waits = list(si.on_wait) if si and si.on_wait else []
                if len(waits) > max_waits:
                    rest = waits[: len(waits) - max_waits]
                    si.on_wait = waits[len(waits) - max_waits :]
                    for i in range(0, len(rest), max_waits):
                        _split_counter[0] += 1
                        nop = mybir.InstNoOp(
                            name=f"I-wsplit-{_split_counter[0]}", ins=[], outs=[]
                        )
                        nop.engine = inst.engine
                        nop.sync_info = type(si)(
                            on_wait=rest[i : i + max_waits], on_update=[]
                        )
                        out.append(nop)
                out.append(inst)
            bb.instructions = out


# ---------------------------------------------------------------------------
# Problem constants
# ---------------------------------------------------------------------------

B, T, C = 2, 2048, 4096
G, QH, D = 8, 4, 128          # kv groups, q heads per group, head dim
S = QH + 2                    # s-blocks per group: q0..q3, k, v
GO = S * D                    # 768 rows of W_attn per group
NT = B * T                    # 4096 tokens
KC = C // 128                 # 32 contraction chunks over C
TT = 256                      # token tile for the QKV matmul
NTT = NT // TT                # 16 token tiles
TQ = 256                      # query tile in attention
TQC = TQ // 128               # q chunks per tile
NJ = T // TQ                  # query tiles per batch
NKC = T // 128                # 16 key chunks per batch
PCH = 4                       # proj contraction chunks (512 = 4*128)
VW = 256                      # padded PV rhs width (129 used)
SCALE = 1.0 / math.sqrt(D)

F32R = mybir.dt.float32r
F32 = mybir.dt.float32
F16 = mybir.dt.float16

N_CORES = 8


def _build_program():
    nc = bass.Bass("TRN2", target_bir_lowering=False, debug=False,
                   num_devices=N_CORES)

    xT = nc.dram_tensor("xT", [KC, 128, NT], F32R, kind="ExternalInput").ap()
    waT = nc.dram_tensor("waT", [KC, 128, GO], F32R, kind="ExternalInput").ap()
    wpT = nc.dram_tensor("wpT", [PCH, 128, C], F32R, kind="ExternalInput").ap()
    cc = nc.dram_tensor("cc", [128, T], F32, kind="ExternalInput").ap()
    ss = nc.dram_tensor("ss", [128, T], F32, kind="ExternalInput").ap()
    tri = nc.dram_tensor("tri", [128, 128], F32R, kind="ExternalInput").ap()
    iden = nc.dram_tensor("iden", [128, 128], F32R, kind="ExternalInput").ap()
    out_p = nc.dram_tensor("out_p", [NT, C], F16, kind="ExternalOutput").ap()

    with TileContext(nc) as tc, ExitStack() as ctx:
        consts = ctx.enter_context(tc.tile_pool(name="consts", bufs=1))
        dram = ctx.enter_context(tc.tile_pool(name="dram", bufs=1, space="DRAM"))

        tri_sb = consts.tile([128, 128], F32R)
        iden_sb = consts.tile([128, 128], F32R)

        # qkv^T scratch in DRAM: [s-block][d][token]
        qkvT = dram.tile([S, 128, NT], F32R)

        # ------------------- Phase 1: QKV projection + RoPE -------------------
        with tc.tile_pool(name="wa", bufs=1) as wap, \
             tc.tile_pool(name="xs", bufs=2) as xsp, \
             tc.tile_pool(name="ropec", bufs=1) as ropec, \
             tc.tile_pool(name="qkps", bufs=4, space="PSUM") as qkps, \
             tc.tile_pool(name="rope", bufs=2) as ropep:
            cc_sb = ropec.tile([128, T], F32)
            ss_sb = ropec.tile([128, T], F32)
            wa_sb = wap.tile([128, KC, GO], F32R)
            # k/v s-blocks first: attention's kT/vT loads depend on the last
            # token-tile's k/v writes, so emit those as early as possible
            M_ORDER = [S - 2, S - 1, 0, 1, 2, 3]

            def _wa_chunk(m, klo=0, khi=KC):
                nc.sync.dma_start(
                    out=wa_sb[:, klo:khi, m * 128 : (m + 1) * 128],
                    in_=waT[klo:khi, :, m * 128 : (m + 1) * 128].rearrange(
                        "k p o -> p k o"
                    ),
                )

            x_tiles = {}

            def _load_x(tt, chunked=False):
                t0 = tt * TT
                xt = xsp.tile([128, KC, TT], F32R, tag="x")
                if chunked:
                    for klo in range(0, KC, 8):
                        nc.sync.dma_start(
                            out=xt[:, klo : klo + 8, :],
                            in_=xT[klo : klo + 8, :, t0 : t0 + TT].rearrange(
                                "k p t -> p k t"
                            ),
                        )
                else:
                    nc.sync.dma_start(
                        out=xt,
                        in_=xT[:, :, t0 : t0 + TT].rearrange("k p t -> p k t"),
                    )
                x_tiles[tt] = xt

            # fine-grained startup: interleave first weight chunk octets with
            # the first x tile octets so matmuls start within a few us and
            # stay fed at the DMA drip rate
            m0 = M_ORDER[0]
            xt0 = xsp.tile([128, KC, TT], F32R, tag="x")
            for klo in range(0, KC, 8):
                _wa_chunk(m0, klo, klo + 8)
                nc.sync.dma_start(
                    out=xt0[:, klo : klo + 8, :],
                    in_=xT[klo : klo + 8, :, 0:TT].rearrange("k p t -> p k t"),
                )
            x_tiles[0] = xt0

            for tt in range(NTT):
                t0 = tt * TT
                x_sb = x_tiles.pop(tt)
                # within-batch column range for the rope tables
                r0 = t0 % T
                for mi, m in enumerate(M_ORDER):
                    ps = qkps.tile([128, TT], F32)
                    for k in range(KC):
                        nc.tensor.matmul(
                            ps,
                            wa_sb[:, k, m * 128 : (m + 1) * 128],
                            x_sb[:, k, :],
                            start=(k == 0),
                            stop=(k == KC - 1),
                        )
                    if mi == 0:
                        if tt == 0:
                            # deferred loads interleaved in PE need-order
                            xt1 = xsp.tile([128, KC, TT], F32R, tag="x")

                            def _x1_oct(klo):
                                nc.sync.dma_start(
                                    out=xt1[:, klo : klo + 8, :],
                                    in_=xT[klo : klo + 8, :, TT : 2 * TT].rearrange(
                                        "k p t -> p k t"
                                    ),
                                )

                            _wa_chunk(M_ORDER[1])
                            _x1_oct(0)
                            _wa_chunk(M_ORDER[2])
                            _x1_oct(8)
                            nc.sync.dma_start(out=cc_sb, in_=cc)
                            nc.sync.dma_start(out=ss_sb, in_=ss)
                            _wa_chunk(M_ORDER[3])
                            _x1_oct(16)
                            _wa_chunk(M_ORDER[4])
                            _x1_oct(24)
                            _wa_chunk(M_ORDER[5])
                            nc.sync.dma_start(out=tri_sb, in_=tri)
                            nc.sync.dma_start(out=iden_sb, in_=iden)
                            x_tiles[1] = xt1
                        elif tt + 1 < NTT:
                            _load_x(tt + 1)
                    if m != S - 1:
                        # RoPE directly from PSUM with partition-offset reads:
                        #   out = ps*CC - rot(ps)*SS, rot = half-swap of ps
                        t1 = ropep.tile([128, TT], F32, tag="t1")
                        nc.vector.tensor_mul(t1, ps, cc_sb[:, r0 : r0 + TT])
                        t2 = ropep.tile([128, TT], F32, tag="t2")
                        nc.vector.tensor_mul(
                            t2[0:64, :], ps[64:128, :], ss_sb[0:64, r0 : r0 + TT]
                        )
                        nc.vector.tensor_mul(
                            t2[64:128, :], ps[0:64, :], ss_sb[64:128, r0 : r0 + TT]
                        )
                        qr = ropep.tile([128, TT], F32R, tag="qr")
                        nc.vector.tensor_sub(qr, t1, t2)
                        nc.gpsimd.dma_start(out=qkvT[m, :, t0 : t0 + TT], in_=qr)
                    else:
                        vr = ropep.tile([128, TT], F32R, tag="qr")
                        nc.scalar.copy(out=vr, in_=ps)
                        nc.gpsimd.dma_start(out=qkvT[m, :, t0 : t0 + TT], in_=vr)

        # --------- Phase 2: fused attention + output projection ---------
        # j-outer / h-inner; after all 4 heads finish a query tile, its
        # output-projection matmuls run against a small rolling y^T buffer,
        # interleaving proj with the next tile's attention on the PE.
        with tc.tile_pool(name="wp", bufs=1) as wpp, \
             tc.tile_pool(name="kv", bufs=1) as kvp, \
             tc.tile_pool(name="qs", bufs=4) as qsp, \
             tc.tile_pool(name="va", bufs=1) as vap, \
             tc.tile_pool(name="e", bufs=2) as ep, \
             tc.tile_pool(name="yr", bufs=3) as yrp, \
             tc.tile_pool(name="sps", bufs=3, space="PSUM") as sps, \
             tc.tile_pool(name="pvps", bufs=2, space="PSUM") as pvps, \
             tc.tile_pool(name="tps", bufs=1, space="PSUM") as tps, \
             tc.tile_pool(name="ops", bufs=2, space="PSUM") as ops, \
             tc.tile_pool(name="osb", bufs=6) as osbp, \
             tc.tile_pool(name="ytmp", bufs=4) as ytmp:
            wp_sb = wpp.tile([128, PCH, C], F32R)
            for b in range(B):
                bt0 = b * T
                kT = kvp.tile([128, T], F32R, tag="kT")
                vT = kvp.tile([128, T], F32R, tag="vT")
                # chunked so the first score/transpose work starts early
                for c4 in range(0, NKC, 4):
                    cs, ce = c4 * 128, (c4 + 4) * 128
                    nc.sync.dma_start(
                        out=kT[:, cs:ce], in_=qkvT[S - 2, :, bt0 + cs : bt0 + ce]
                    )
                    nc.sync.dma_start(
                        out=vT[:, cs:ce], in_=qkvT[S - 1, :, bt0 + cs : bt0 + ce]
                    )

                # V_aug: [tk 128, chunk, 0:128 v / 128 ones / pad]
                va = vap.tile([128, NKC, VW], F32R)
                nc.vector.memset(va[:, :, 128:130].bitcast(F32), 1.0)
                if VW > 130:
                    nc.vector.memset(va[:, :, 130:VW].bitcast(F32), 0.0)
                for ck in range(NKC):
                    pt = tps.tile([128, 128], F32R, tag="tp")
                    nc.tensor.transpose(
                        pt, vT[:, ck * 128 : (ck + 1) * 128], iden_sb
                    )
                    nc.scalar.copy(out=va[:, ck, 0:128], in_=pt)

                # descending j: the largest query tiles first, so the PE has
                # deep attention work queued while the wp load is in flight
                for j in range(NJ - 1, -1, -1):
                    q0 = j * TQ
                    ncks = TQC * (j + 1)  # key chunks 0 .. TQC*(j+1)-1
                    yroll = yrp.tile([128, QH, TQ], F32R)
                    for h in range(QH):
                        qsl = qsp.tile([128, TQ], F32R)
                        nc.sync.dma_start(
                            out=qsl, in_=qkvT[h, :, bt0 + q0 : bt0 + q0 + TQ]
                        )
                        if b == 0 and j == NJ - 1:
                            # wp is first needed once the first query tile's
                            # attention completes; drip one chunk per head so
                            # the q slices are not stuck behind it in the queue
                            nc.sync.dma_start(out=wp_sb[:, h, :], in_=wpT[h])
                        e_sb = ep.tile([128, NKC, TQ], F32R)
                        for ck in range(ncks):
                            ps = sps.tile([128, TQ], F32, tag="s")
                            nc.tensor.matmul(
                                ps,
                                kT[:, ck * 128 : (ck + 1) * 128],
                                qsl,
                                start=True,
                                stop=True,
                            )
                            c = ck - TQC * j
                            lo = 0 if c < 0 else c * 128
                            nc.scalar.activation(
                                out=e_sb[:, ck, lo:TQ],
                                in_=ps[:, lo:TQ],
                                func=mybir.ActivationFunctionType.Exp,
                                scale=SCALE,
                            )
                            if c >= 0:
                                nc.vector.tensor_mul(
                                    e_sb[:, ck, lo : lo + 128],
                                    e_sb[:, ck, lo : lo + 128],
                                    tri_sb,
                                )
                        for cq in range(TQC):
                            qc = TQC * j + cq  # global q chunk
                            pv = pvps.tile([128, VW], F32, tag="pv")
                            for ck in range(qc + 1):
                                nc.tensor.matmul(
                                    pv,
                                    e_sb[:, ck, cq * 128 : (cq + 1) * 128],
                                    va[:, ck, :],
                                    start=(ck == 0),
                                    stop=(ck == qc),
                                )
                            rcp = ytmp.tile([128, 1], F32, tag="rcp")
                            nc.vector.reciprocal(rcp, pv[:, 128:129])
                            ysb = ytmp.tile([128, 128], F32R, tag="ysb")
                            nc.vector.tensor_scalar_mul(ysb, pv[:, 0:128], rcp)
                            yt_ps = tps.tile([128, 128], F32R, tag="tp")
                            nc.tensor.transpose(yt_ps, ysb, iden_sb)
                            nc.scalar.copy(
                                out=yroll[:, h, cq * 128 : (cq + 1) * 128],
                                in_=yt_ps,
                            )
                    # output projection for this (b, j) query tile
                    for cq in range(TQC):
                        trow = bt0 + q0 + cq * 128
                        for ot in range(C // 512):
                            po = ops.tile([128, 512], F32)
                            for h in range(PCH):
                                nc.tensor.matmul(
                                    po,
                                    yroll[:, h, cq * 128 : (cq + 1) * 128],
                                    wp_sb[:, h, ot * 512 : (ot + 1) * 512],
                                    start=(h == 0),
                                    stop=(h == PCH - 1),
                                )
                            o_sb = osbp.tile([128, 512], F16)
                            if ot % 2 == 0:
                                nc.scalar.copy(out=o_sb, in_=po)
                            else:
                                nc.vector.tensor_copy(out=o_sb, in_=po)
                            nc.gpsimd.dma_start(
                                out=out_p[
                                    trow : trow + 128, ot * 512 : (ot + 1) * 512
                                ],
                                in_=o_sb,
                            )

    _split_multi_waits(nc)
    return nc


_PROGRAM = None


def _get_program():
    global _PROGRAM
    if _PROGRAM is None:
        _PROGRAM = _build_program()
    return _PROGRAM


def _prep_inputs(x, cos, sin, W_attn, W_proj):
    x = np.ascontiguousarray(np.asarray(x, dtype=np.float32))
    cos = np.asarray(cos, dtype=np.float32)
    sin = np.asarray(sin, dtype=np.float32)
    W_attn = np.asarray(W_attn, dtype=np.float32)
    W_proj = np.asarray(W_proj, dtype=np.float32)

    xT = np.ascontiguousarray(
        x.reshape(NT, C).T.reshape(KC, 128, NT)
    )
    cosT = cos.T  # (64, T)
    sinT = sin.T
    cc = np.ascontiguousarray(np.concatenate([cosT, cosT], axis=0))
    ss = np.ascontiguousarray(np.concatenate([sinT, -sinT], axis=0))
    ii, jj = np.meshgrid(np.arange(128), np.arange(128), indexing="ij")
    tri = (ii <= jj).astype(np.float32)
    iden = np.eye(128, dtype=np.float32)

    in_maps = []
    for g in range(N_CORES):
        waT = np.ascontiguousarray(
            W_attn[g * GO : (g + 1) * GO, :].T.reshape(KC, 128, GO)
        )
        wpT = np.ascontiguousarray(
            W_proj[:, g * 512 : (g + 1) * 512].T.reshape(PCH, 128, C)
        )
        in_maps.append(
            {
                "xT": xT,
                "waT": waT,
                "wpT": wpT,
                "cc": cc,
                "ss": ss,
                "tri": tri,
                "iden": iden,
            }
        )
    return in_maps


def kernel(x, cos, sin, W_attn, W_proj, _return_results=False, **trace_kwargs):
    nc = _get_program()
    in_maps = _prep_inputs(x, cos, sin, W_attn, W_proj)
    res = run_bass_kernel_spmd(nc, in_maps, list(range(N_CORES)), **trace_kwargs)
    acc = res.results[0]["out_p"].astype(np.float32)
    for g in range(1, N_CORES):
        acc = acc + res.results[g]["out_p"].astype(np.float32)
    out = acc.reshape(B, T, C)
    if _return_results:
        return out, res
    return out


# revision 38
# speedup vs baseline: 1.0072x; 1.0072x over previous
"""Causal self-attention (GQA) Trainium2 Bass kernel.

Config (hardcoded): B=2, T=2048, C=4096, n_head=32, n_query_groups=8,
head_size=128, qkv_out=6144. RoPE (NeoX, rotary dim = head_size).

Sharding: tensor-parallel over the 8 KV groups, one group per NeuronCore.
Each core computes its group's QKV projection, RoPE, causal attention for
its 4 query heads, and a partial output projection against its 512 columns
of W_proj. The 8 fp32 partials are summed on the host (the all-reduce is
host-side since full outputs are gathered anyway).

Everything on the PE runs in float32r (fp32 storage, PE rounds to its
fast-fp32 format): ~1.5e-4 matmul rel err vs 2.3e-3 for bf16, at full PE
rate when the moving dim is >=256.

Attention is computed in the S^T = K@Q^T orientation so the softmax
reduction lands on the free axis sum via a ones-column in the PV matmul,
and no P-transposes are needed (only V and the final per-head Y tiles get
PE transposes).
"""

import math
from contextlib import ExitStack

import numpy as np

import concourse.bass as bass
import concourse.tile as tile
from concourse import mybir
from concourse.bass_utils import run_bass_kernel_spmd
from concourse.tile import TileContext
from concourse.vector_clock import ScopedClock

# ---------------------------------------------------------------------------
# Workarounds: walrus codegen on this toolchain rejects >1 sync wait per
# instruction ("Too many sync wait commands"). Split excess waits onto
# extra SP drains (TileContext tail) / same-engine InstNoOp instructions.
# ---------------------------------------------------------------------------

_MAX_WAITS = 1


def _patched_drain_and_barrier(self, tick_clock, wait_clock):
    drain_inst = self.nc.sync.drain()
    wait_clock.add_sem_waits(
        drain_inst.ins, ScopedClock({None: tick_clock.global_clock})
    )
    si = drain_inst.ins.sync_info
    waits = list(si.on_wait or [])
    if len(waits) > _MAX_WAITS:
        si.on_wait = waits[:_MAX_WAITS]
        rest = waits[_MAX_WAITS:]
        for i in range(0, len(rest), _MAX_WAITS):
            extra = self.nc.sync.drain()
            extra.ins.sync_info = type(si)(
                on_wait=rest[i : i + _MAX_WAITS], on_update=[]
            )

    self.nc.all_engine_barrier()
    assert self.sems is not None
    popped = self.nc._tile_sem_poison_stack.pop()
    assert popped is self._sem_poison
    self.nc.clear_and_free_semaphores(list(self.sems.allocated().values()))
    self.nc.all_engine_barrier()


tile.TileContext._drain_and_barrier = _patched_drain_and_barrier

_split_counter = [0]


def _split_multi_waits(nc, max_waits=_MAX_WAITS):
    for fn in nc.m.functions:
        for bb in fn.blocks:
            out = []
            for inst in bb.instructions:
                si = inst.sync_info
                waits = list(si.on_wait) if si and si.on_wait else []
                if len(waits) > max_waits:
                    rest = waits[: len(waits) - max_waits]
                    si.on_wait = waits[len(waits) - max_waits :]
                    for i in range(0, len(rest), max_waits):
                        _split_counter[0] += 1
                        nop = mybir.InstNoOp(
                            name=f"I-wsplit-{_split_counter[0]}", ins=[], outs=[]
                        )
                        nop.engine = inst.engine
                        nop.sync_info = type(si)(
                            on_wait=rest[i : i + max_waits], on_update=[]
                        )
                        out.append(nop)
                out.append(inst)
            bb.instructions = out


# ---------------------------------------------------------------------------
# Problem constants
# ---------------------------------------------------------------------------

B, T, C = 2, 2048, 4096
G, QH, D = 8, 4, 128          # kv groups, q heads per group, head dim
S = QH + 2                    # s-blocks per group: q0..q3, k, v
GO = S * D                    # 768 rows of W_attn per group
NT = B * T                    # 4096 tokens
KC = C // 128                 # 32 contraction chunks over C
TT = 256                      # token tile for the QKV matmul
NTT = NT // TT                # 16 token tiles
TQ = 256                      # query tile in attention
TQC = TQ // 128               # q chunks per tile
NJ = T // TQ                  # query tiles per batch
NKC = T // 128                # 16 key chunks per batch
PCH = 4                       # proj contraction chunks (512 = 4*128)
VW = 132                      # PV rhs width (129 used; f16 runs 1cyc/row at any width)
SCALE = 1.0 / math.sqrt(D)
EXP_BIAS = -9.0

F32R = mybir.dt.float32r
F32 = mybir.dt.float32
F16 = mybir.dt.float16

N_CORES = 8


def _build_program():
    nc = bass.Bass("TRN2", target_bir_lowering=False, debug=False,
                   num_devices=N_CORES)

    xT = nc.dram_tensor("xT", [KC, 128, NT], F32R, kind="ExternalInput").ap()
    waT = nc.dram_tensor("waT", [KC, 128, GO], F32R, kind="ExternalInput").ap()
    wpT = nc.dram_tensor("wpT", [PCH, 128, C], F32R, kind="ExternalInput").ap()
    cc = nc.dram_tensor("cc", [128, T], F32, kind="ExternalInput").ap()
    ss = nc.dram_tensor("ss", [128, T], F32, kind="ExternalInput").ap()
    tri = nc.dram_tensor("tri", [128, 128], F16, kind="ExternalInput").ap()
    iden = nc.dram_tensor("iden", [128, 128], F32R, kind="ExternalInput").ap()
    out_p = nc.dram_tensor("out_p", [NT, C], F16, kind="ExternalOutput").ap()

    with TileContext(nc) as tc, ExitStack() as ctx:
        consts = ctx.enter_context(tc.tile_pool(name="consts", bufs=1))
        dram = ctx.enter_context(tc.tile_pool(name="dram", bufs=1, space="DRAM"))

        tri_sb = consts.tile([128, 128], F16)
        iden_sb = consts.tile([128, 128], F32R)
        ebias_sb = consts.tile([128, 1], F32)
        nc.vector.memset(ebias_sb, EXP_BIAS)

        # qkv^T scratch in DRAM: [s-block][d][token]
        qkvT = dram.tile([S, 128, NT], F32R)

        # ------------------- Phase 1: QKV projection + RoPE -------------------
        with tc.tile_pool(name="wa", bufs=1) as wap, \
             tc.tile_pool(name="xs", bufs=2) as xsp, \
             tc.tile_pool(name="ropec", bufs=1) as ropec, \
             tc.tile_pool(name="qkps", bufs=4, space="PSUM") as qkps, \
             tc.tile_pool(name="rope", bufs=2) as ropep:
            cc_sb = ropec.tile([128, T], F32)
            ss_sb = ropec.tile([128, T], F32)
            wa_sb = wap.tile([128, KC, GO], F32R)
            # k/v s-blocks first: attention's kT/vT loads depend on the last
            # token-tile's k/v writes, so emit those as early as possible
            M_ORDER = [S - 2, S - 1, 0, 1, 2, 3]

            def _wa_chunk(m, klo=0, khi=KC):
                nc.sync.dma_start(
                    out=wa_sb[:, klo:khi, m * 128 : (m + 1) * 128],
                    in_=waT[klo:khi, :, m * 128 : (m + 1) * 128].rearrange(
                        "k p o -> p k o"
                    ),
                )

            x_tiles = {}

            def _load_x(tt, chunked=False):
                t0 = tt * TT
                xt = xsp.tile([128, KC, TT], F32R, tag="x")
                if chunked:
                    for klo in range(0, KC, 8):
                        nc.sync.dma_start(
                            out=xt[:, klo : klo + 8, :],
                            in_=xT[klo : klo + 8, :, t0 : t0 + TT].rearrange(
                                "k p t -> p k t"
                            ),
                        )
                else:
                    nc.sync.dma_start(
                        out=xt,
                        in_=xT[:, :, t0 : t0 + TT].rearrange("k p t -> p k t"),
                    )
                x_tiles[tt] = xt

            # fine-grained startup: interleave first weight chunk octets with
            # the first x tile octets so matmuls start within a few us and
            # stay fed at the DMA drip rate
            m0 = M_ORDER[0]
            xt0 = xsp.tile([128, KC, TT], F32R, tag="x")
            for klo in range(0, KC, 8):
                _wa_chunk(m0, klo, klo + 8)
                nc.sync.dma_start(
                    out=xt0[:, klo : klo + 8, :],
                    in_=xT[klo : klo + 8, :, 0:TT].rearrange("k p t -> p k t"),
                )
            x_tiles[0] = xt0

            for tt in range(NTT):
                t0 = tt * TT
                x_sb = x_tiles.pop(tt)
                # within-batch column range for the rope tables
                r0 = t0 % T
                for mi, m in enumerate(M_ORDER):
                    ps = qkps.tile([128, TT], F32)
                    for k in range(KC):
                        nc.tensor.matmul(
                            ps,
                            wa_sb[:, k, m * 128 : (m + 1) * 128],
                            x_sb[:, k, :],
                            start=(k == 0),
                            stop=(k == KC - 1),
                        )
                    if mi == 0:
                        if tt == 0:
                            # deferred loads interleaved in PE need-order
                            xt1 = xsp.tile([128, KC, TT], F32R, tag="x")

                            def _x1_oct(klo):
                                nc.sync.dma_start(
                                    out=xt1[:, klo : klo + 8, :],
                                    in_=xT[klo : klo + 8, :, TT : 2 * TT].rearrange(
                                        "k p t -> p k t"
                                    ),
                                )

                            _wa_chunk(M_ORDER[1])
                            _x1_oct(0)
                            _wa_chunk(M_ORDER[2])
                            _x1_oct(8)
                            nc.sync.dma_start(out=cc_sb, in_=cc)
                            nc.sync.dma_start(out=ss_sb, in_=ss)
                            _wa_chunk(M_ORDER[3])
                            _x1_oct(16)
                            _wa_chunk(M_ORDER[4])
                            _x1_oct(24)
                            _wa_chunk(M_ORDER[5])
                            nc.sync.dma_start(out=tri_sb, in_=tri)
                            nc.sync.dma_start(out=iden_sb, in_=iden)
                            x_tiles[1] = xt1
                        elif tt + 1 < NTT:
                            _load_x(tt + 1)
                    if m != S - 1:
                        # RoPE directly from PSUM with partition-offset reads:
                        #   out = ps*CC - rot(ps)*SS, rot = half-swap of ps
                        t1 = ropep.tile([128, TT], F32, tag="t1")
                        nc.vector.tensor_mul(t1, ps, cc_sb[:, r0 : r0 + TT])
                        t2 = ropep.tile([128, TT], F32, tag="t2")
                        nc.vector.tensor_mul(
                            t2[0:64, :], ps[64:128, :], ss_sb[0:64, r0 : r0 + TT]
                        )
                        nc.vector.tensor_mul(
                            t2[64:128, :], ps[0:64, :], ss_sb[64:128, r0 : r0 + TT]
                        )
                        qr = ropep.tile([128, TT], F32R, tag="qr")
                        nc.vector.tensor_sub(qr, t1, t2)
                        nc.gpsimd.dma_start(out=qkvT[m, :, t0 : t0 + TT], in_=qr)
                    else:
                        vr = ropep.tile([128, TT], F32R, tag="qr")
                        nc.scalar.copy(out=vr, in_=ps)
                        nc.gpsimd.dma_start(out=qkvT[m, :, t0 : t0 + TT], in_=vr)

        # --------- Phase 2: fused attention + output projection ---------
        # j-outer / h-inner; after all 4 heads finish a query tile, its
        # output-projection matmuls run against a small rolling y^T buffer,
        # interleaving proj with the next tile's attention on the PE.
        with tc.tile_pool(name="wp", bufs=1) as wpp, \
             tc.tile_pool(name="kv", bufs=1) as kvp, \
             tc.tile_pool(name="qs", bufs=4) as qsp, \
             tc.tile_pool(name="va", bufs=1) as vap, \
             tc.tile_pool(name="e", bufs=2) as ep, \
             tc.tile_pool(name="yr", bufs=3) as yrp, \
             tc.tile_pool(name="sps", bufs=3, space="PSUM") as sps, \
             tc.tile_pool(name="pvps", bufs=2, space="PSUM") as pvps, \
             tc.tile_pool(name="tps", bufs=1, space="PSUM") as tps, \
             tc.tile_pool(name="ops", bufs=2, space="PSUM") as ops, \
             tc.tile_pool(name="osb", bufs=6) as osbp, \
             tc.tile_pool(name="ytmp", bufs=4) as ytmp:
            wp_sb = wpp.tile([128, PCH, C], F32R)
            for b in range(B):
                bt0 = b * T
                kT = kvp.tile([128, T], F32R, tag="kT")
                vT = kvp.tile([128, T], F32R, tag="vT")
                # chunked so the first score/transpose work starts early
                for c4 in range(0, NKC, 4):
                    cs, ce = c4 * 128, (c4 + 4) * 128
                    nc.sync.dma_start(
                        out=kT[:, cs:ce], in_=qkvT[S - 2, :, bt0 + cs : bt0 + ce]
                    )
                    nc.sync.dma_start(
                        out=vT[:, cs:ce], in_=qkvT[S - 1, :, bt0 + cs : bt0 + ce]
                    )

                # V_aug: [tk 128, chunk, 0:128 v / 128 ones / pad]
                va = vap.tile([128, NKC, VW], F16)
                nc.vector.memset(va[:, :, 128:130], 1.0)
                if VW > 130:
                    nc.vector.memset(va[:, :, 130:VW], 0.0)
                for ck in range(NKC):
                    pt = tps.tile([128, 128], F32R, tag="tp")
                    nc.tensor.transpose(
                        pt, vT[:, ck * 128 : (ck + 1) * 128], iden_sb
                    )
                    nc.scalar.copy(out=va[:, ck, 0:128], in_=pt)

                # descending j: the largest query tiles first, so the PE has
                # deep attention work queued while the wp load is in flight
                for j in range(NJ - 1, -1, -1):
                    q0 = j * TQ
                    ncks = TQC * (j + 1)  # key chunks 0 .. TQC*(j+1)-1
                    yroll = yrp.tile([128, QH, TQ], F32R)
                    for h in range(QH):
                        qsl = qsp.tile([128, TQ], F32R)
                        nc.sync.dma_start(
                            out=qsl, in_=qkvT[h, :, bt0 + q0 : bt0 + q0 + TQ]
                        )
                        if b == 0 and j == NJ - 1:
                            # wp is first needed once the first query tile's
                            # attention completes; drip one chunk per head so
                            # the q slices are not stuck behind it in the queue
                            nc.sync.dma_start(out=wp_sb[:, h, :], in_=wpT[h])
                        e_sb = ep.tile([128, NKC, TQ], F16)
                        for ck in range(ncks):
                            ps = sps.tile([128, TQ], F32, tag="s")
                            nc.tensor.matmul(
                                ps,
                                kT[:, ck * 128 : (ck + 1) * 128],
                                qsl,
                                start=True,
                                stop=True,
                            )
                            c = ck - TQC * j
                            lo = 0 if c < 0 else c * 128
                            nc.scalar.activation(
                                out=e_sb[:, ck, lo:TQ],
                                in_=ps[:, lo:TQ],
                                func=mybir.ActivationFunctionType.Exp,
                                scale=SCALE,
                                # keeps E within f16 range; the common factor
                                # cancels exactly in the softmax division
                                bias=ebias_sb,
                            )
                            if c >= 0:
                                nc.vector.tensor_mul(
                                    e_sb[:, ck, lo : lo + 128],
                                    e_sb[:, ck, lo : lo + 128],
                                    tri_sb,
                                )
                        for cq in range(TQC):
                            qc = TQC * j + cq  # global q chunk
                            pv = pvps.tile([128, VW], F32, tag="pv")
                            for ck in range(qc + 1):
                                nc.tensor.matmul(
                                    pv,
                                    e_sb[:, ck, cq * 128 : (cq + 1) * 128],
                                    va[:, ck, :],
                                    start=(ck == 0),
                                    stop=(ck == qc),
                                )
                            rcp = ytmp.tile([128, 1], F32, tag="rcp")
                            nc.vector.reciprocal(rcp, pv[:, 128:129])
                            ysb = ytmp.tile([128, 128], F32R, tag="ysb")
                            nc.vector.tensor_scalar_mul(ysb, pv[:, 0:128], rcp)
                            yt_ps = tps.tile([128, 128], F32R, tag="tp")
                            nc.tensor.transpose(yt_ps, ysb, iden_sb)
                            nc.scalar.copy(
                                out=yroll[:, h, cq * 128 : (cq + 1) * 128],
                                in_=yt_ps,
                            )
                    # output projection for this (b, j) query tile
                    for cq in range(TQC):
                        trow = bt0 + q0 + cq * 128
                        for ot in range(C // 512):
                            po = ops.tile([128, 512], F32)
                            for h in range(PCH):
                                nc.tensor.matmul(
                                    po,
                                    yroll[:, h, cq * 128 : (cq + 1) * 128],
                                    wp_sb[:, h, ot * 512 : (ot + 1) * 512],
                                    start=(h == 0),
                                    stop=(h == PCH - 1),
                                )
                            o_sb = osbp.tile([128, 512], F16)
                            if ot % 2 == 0:
                                nc.scalar.copy(out=o_sb, in_=po)
                            else:
                                nc.vector.tensor_copy(out=o_sb, in_=po)
                            nc.gpsimd.dma_start(
                                out=out_p[
                                    trow : trow + 128, ot * 512 : (ot + 1) * 512
                                ],
                                in_=o_sb,
                            )

    _split_multi_waits(nc)
    return nc


_PROGRAM = None


def _get_program():
    global _PROGRAM
    if _PROGRAM is None:
        _PROGRAM = _build_program()
    return _PROGRAM


def _prep_inputs(x, cos, sin, W_attn, W_proj):
    x = np.ascontiguousarray(np.asarray(x, dtype=np.float32))
    cos = np.asarray(cos, dtype=np.float32)
    sin = np.asarray(sin, dtype=np.float32)
    W_attn = np.asarray(W_attn, dtype=np.float32)
    W_proj = np.asarray(W_proj, dtype=np.float32)

    xT = np.ascontiguousarray(
        x.reshape(NT, C).T.reshape(KC, 128, NT)
    )
    cosT = cos.T  # (64, T)
    sinT = sin.T
    cc = np.ascontiguousarray(np.concatenate([cosT, cosT], axis=0))
    ss = np.ascontiguousarray(np.concatenate([sinT, -sinT], axis=0))
    ii, jj = np.meshgrid(np.arange(128), np.arange(128), indexing="ij")
    tri = (ii <= jj).astype(np.float16)
    iden = np.eye(128, dtype=np.float32)

    in_maps = []
    for g in range(N_CORES):
        waT = np.ascontiguousarray(
            W_attn[g * GO : (g + 1) * GO, :].T.reshape(KC, 128, GO)
        )
        wpT = np.ascontiguousarray(
            W_proj[:, g * 512 : (g + 1) * 512].T.reshape(PCH, 128, C)
        )
        in_maps.append(
            {
                "xT": xT,
                "waT": waT,
                "wpT": wpT,
                "cc": cc,
                "ss": ss,
                "tri": tri,
                "iden": iden,
            }
        )
    return in_maps


def kernel(x, cos, sin, W_attn, W_proj, _return_results=False, **trace_kwargs):
    nc = _get_program()
    in_maps = _prep_inputs(x, cos, sin, W_attn, W_proj)
    res = run_bass_kernel_spmd(nc, in_maps, list(range(N_CORES)), **trace_kwargs)
    acc = res.results[0]["out_p"].astype(np.float32)
    for g in range(1, N_CORES):
        acc = acc + res.results[g]["out_p"].astype(np.float32)
    out = acc.reshape(B, T, C)
    if _return_results:
        return out, res
    return out


# revision 42
# speedup vs baseline: 1.0533x; 1.0457x over previous
"""Causal self-attention (GQA) Trainium2 Bass kernel.

Config (hardcoded): B=2, T=2048, C=4096, n_head=32, n_query_groups=8,
head_size=128, qkv_out=6144. RoPE (NeoX, rotary dim = head_size).

Sharding: tensor-parallel over the 8 KV groups, one group per NeuronCore.
Each core computes its group's QKV projection, RoPE, causal attention for
its 4 query heads, and a partial output projection against its 512 columns
of W_proj. The 8 fp32 partials are summed on the host (the all-reduce is
host-side since full outputs are gathered anyway).

Everything on the PE runs in float32r (fp32 storage, PE rounds to its
fast-fp32 format): ~1.5e-4 matmul rel err vs 2.3e-3 for bf16, at full PE
rate when the moving dim is >=256.

Attention is computed in the S^T = K@Q^T orientation so the softmax
reduction lands on the free axis sum via a ones-column in the PV matmul,
and no P-transposes are needed (only V and the final per-head Y tiles get
PE transposes).
"""

import math
from contextlib import ExitStack

import numpy as np

import concourse.bass as bass
import concourse.tile as tile
from concourse import mybir
from concourse.bass_utils import run_bass_kernel_spmd
from concourse.tile import TileContext
from concourse.vector_clock import ScopedClock

# ---------------------------------------------------------------------------
# Workarounds: walrus codegen on this toolchain rejects >1 sync wait per
# instruction ("Too many sync wait commands"). Split excess waits onto
# extra SP drains (TileContext tail) / same-engine InstNoOp instructions.
# ---------------------------------------------------------------------------

_MAX_WAITS = 1


def _patched_drain_and_barrier(self, tick_clock, wait_clock):
    drain_inst = self.nc.sync.drain()
    wait_clock.add_sem_waits(
        drain_inst.ins, ScopedClock({None: tick_clock.global_clock})
    )
    si = drain_inst.ins.sync_info
    waits = list(si.on_wait or [])
    if len(waits) > _MAX_WAITS:
        si.on_wait = waits[:_MAX_WAITS]
        rest = waits[_MAX_WAITS:]
        for i in range(0, len(rest), _MAX_WAITS):
            extra = self.nc.sync.drain()
            extra.ins.sync_info = type(si)(
                on_wait=rest[i : i + _MAX_WAITS], on_update=[]
            )

    self.nc.all_engine_barrier()
    assert self.sems is not None
    popped = self.nc._tile_sem_poison_stack.pop()
    assert popped is self._sem_poison
    self.nc.clear_and_free_semaphores(list(self.sems.allocated().values()))
    self.nc.all_engine_barrier()


tile.TileContext._drain_and_barrier = _patched_drain_and_barrier

_split_counter = [0]


def _split_multi_waits(nc, max_waits=_MAX_WAITS):
    for fn in nc.m.functions:
        for bb in fn.blocks:
            out = []
            for inst in bb.instructions:
                si = inst.sync_info
                waits = list(si.on_wait) if si and si.on_wait else []
                if len(waits) > max_waits:
                    rest = waits[: len(waits) - max_waits]
                    si.on_wait = waits[len(waits) - max_waits :]
                    for i in range(0, len(rest), max_waits):
                        _split_counter[0] += 1
                        nop = mybir.InstNoOp(
                            name=f"I-wsplit-{_split_counter[0]}", ins=[], outs=[]
                        )
                        nop.engine = inst.engine
                        nop.sync_info = type(si)(
                            on_wait=rest[i : i + max_waits], on_update=[]
                        )
                        out.append(nop)
                out.append(inst)
            bb.instructions = out


# ---------------------------------------------------------------------------
# Problem constants
# ---------------------------------------------------------------------------

B, T, C = 2, 2048, 4096
G, QH, D = 8, 4, 128          # kv groups, q heads per group, head dim
S = QH + 2                    # s-blocks per group: q0..q3, k, v
GO = S * D                    # 768 rows of W_attn per group
NT = B * T                    # 4096 tokens
KC = C // 128                 # 32 contraction chunks over C
TT = 256                      # token tile for the QKV matmul
NTT = NT // TT                # 16 token tiles
TQ = 256                      # query tile in attention
TQC = TQ // 128               # q chunks per tile
NJ = T // TQ                  # query tiles per batch
NKC = T // 128                # 16 key chunks per batch
PCH = 4                       # proj contraction chunks (512 = 4*128)
VW = 132                      # PV rhs width (129 used; f16 runs 1cyc/row at any width)
SCALE = 1.0 / math.sqrt(D)
EXP_BIAS = -9.0

F32R = mybir.dt.float32r
F32 = mybir.dt.float32
F16 = mybir.dt.float16

N_CORES = 8


def _build_program():
    nc = bass.Bass("TRN2", target_bir_lowering=False, debug=False,
                   num_devices=N_CORES)

    xT = nc.dram_tensor("xT", [KC, 128, NT], F32R, kind="ExternalInput").ap()
    waT = nc.dram_tensor("waT", [KC, 128, GO], F32R, kind="ExternalInput").ap()
    wpT = nc.dram_tensor("wpT", [PCH, 128, C], F32R, kind="ExternalInput").ap()
    cc = nc.dram_tensor("cc", [128, T], F32, kind="ExternalInput").ap()
    ss = nc.dram_tensor("ss", [128, T], F32, kind="ExternalInput").ap()
    tri = nc.dram_tensor("tri", [128, 128], F16, kind="ExternalInput").ap()
    iden = nc.dram_tensor("iden", [128, 128], F32R, kind="ExternalInput").ap()
    out_p = nc.dram_tensor("out_p", [NT, C], F16, kind="ExternalOutput").ap()

    with TileContext(nc) as tc, ExitStack() as ctx:
        consts = ctx.enter_context(tc.tile_pool(name="consts", bufs=1))
        dram = ctx.enter_context(tc.tile_pool(name="dram", bufs=1, space="DRAM"))

        tri_sb = consts.tile([128, 128], F16)
        iden_sb = consts.tile([128, 128], F32R)
        ebias_sb = consts.tile([128, 1], F32)
        nc.vector.memset(ebias_sb, EXP_BIAS)

        # qkv^T scratch in DRAM: [s-block][d][token]
        qkvT = dram.tile([S, 128, NT], F32R)

        # ------------------- Phase 1: QKV projection + RoPE -------------------
        with tc.tile_pool(name="wa", bufs=1) as wap, \
             tc.tile_pool(name="xs", bufs=2) as xsp, \
             tc.tile_pool(name="ropec", bufs=1) as ropec, \
             tc.tile_pool(name="qkps", bufs=4, space="PSUM") as qkps, \
             tc.tile_pool(name="rope", bufs=2) as ropep:
            cc_sb = ropec.tile([128, T], F32)
            ss_sb = ropec.tile([128, T], F32)
            wa_sb = wap.tile([128, KC, GO], F32R)
            # k/v s-blocks first: attention's kT/vT loads depend on the last
            # token-tile's k/v writes, so emit those as early as possible
            M_ORDER = [S - 2, S - 1, 0, 1, 2, 3]

            def _wa_chunk(m, klo=0, khi=KC):
                nc.sync.dma_start(
                    out=wa_sb[:, klo:khi, m * 128 : (m + 1) * 128],
                    in_=waT[klo:khi, :, m * 128 : (m + 1) * 128].rearrange(
                        "k p o -> p k o"
                    ),
                )

            x_tiles = {}

            def _load_x(tt, chunked=False):
                t0 = tt * TT
                xt = xsp.tile([128, KC, TT], F32R, tag="x")
                if chunked:
                    for klo in range(0, KC, 8):
                        nc.sync.dma_start(
                            out=xt[:, klo : klo + 8, :],
                            in_=xT[klo : klo + 8, :, t0 : t0 + TT].rearrange(
                                "k p t -> p k t"
                            ),
                        )
                else:
                    nc.sync.dma_start(
                        out=xt,
                        in_=xT[:, :, t0 : t0 + TT].rearrange("k p t -> p k t"),
                    )
                x_tiles[tt] = xt

            # fine-grained startup: interleave first weight chunk octets with
            # the first x tile octets so matmuls start within a few us and
            # stay fed at the DMA drip rate
            m0 = M_ORDER[0]
            xt0 = xsp.tile([128, KC, TT], F32R, tag="x")
            for klo in range(0, KC, 8):
                _wa_chunk(m0, klo, klo + 8)
                nc.sync.dma_start(
                    out=xt0[:, klo : klo + 8, :],
                    in_=xT[klo : klo + 8, :, 0:TT].rearrange("k p t -> p k t"),
                )
            x_tiles[0] = xt0

            for tt in range(NTT):
                t0 = tt * TT
                x_sb = x_tiles.pop(tt)
                # within-batch column range for the rope tables
                r0 = t0 % T
                for mi, m in enumerate(M_ORDER):
                    ps = qkps.tile([128, TT], F32)
                    for k in range(KC):
                        nc.tensor.matmul(
                            ps,
                            wa_sb[:, k, m * 128 : (m + 1) * 128],
                            x_sb[:, k, :],
                            start=(k == 0),
                            stop=(k == KC - 1),
                        )
                    if mi == 0:
                        if tt == 0:
                            # deferred loads interleaved in PE need-order
                            xt1 = xsp.tile([128, KC, TT], F32R, tag="x")

                            def _x1_oct(klo):
                                nc.sync.dma_start(
                                    out=xt1[:, klo : klo + 8, :],
                                    in_=xT[klo : klo + 8, :, TT : 2 * TT].rearrange(
                                        "k p t -> p k t"
                                    ),
                                )

                            _wa_chunk(M_ORDER[1])
                            _x1_oct(0)
                            _wa_chunk(M_ORDER[2])
                            _x1_oct(8)
                            nc.sync.dma_start(out=cc_sb, in_=cc)
                            nc.sync.dma_start(out=ss_sb, in_=ss)
                            _wa_chunk(M_ORDER[3])
                            _x1_oct(16)
                            _wa_chunk(M_ORDER[4])
                            _x1_oct(24)
                            _wa_chunk(M_ORDER[5])
                            nc.sync.dma_start(out=tri_sb, in_=tri)
                            nc.sync.dma_start(out=iden_sb, in_=iden)
                            x_tiles[1] = xt1
                        elif tt + 1 < NTT:
                            _load_x(tt + 1)
                    if m != S - 1:
                        # RoPE directly from PSUM with partition-offset reads:
                        #   out = ps*CC - rot(ps)*SS, rot = half-swap of ps
                        t1 = ropep.tile([128, TT], F32, tag="t1")
                        nc.vector.tensor_mul(t1, ps, cc_sb[:, r0 : r0 + TT])
                        t2 = ropep.tile([128, TT], F32, tag="t2")
                        nc.vector.tensor_mul(
                            t2[0:64, :], ps[64:128, :], ss_sb[0:64, r0 : r0 + TT]
                        )
                        nc.vector.tensor_mul(
                            t2[64:128, :], ps[0:64, :], ss_sb[64:128, r0 : r0 + TT]
                        )
                        qr = ropep.tile([128, TT], F32R, tag="qr")
                        nc.vector.tensor_sub(qr, t1, t2)
                        nc.gpsimd.dma_start(out=qkvT[m, :, t0 : t0 + TT], in_=qr)
                    else:
                        vr = ropep.tile([128, TT], F32R, tag="qr")
                        nc.scalar.copy(out=vr, in_=ps)
                        nc.gpsimd.dma_start(out=qkvT[m, :, t0 : t0 + TT], in_=vr)

        # --------- Phase 2: fused attention + output projection ---------
        # j-outer / h-inner; after all 4 heads finish a query tile, its
        # output-projection matmuls run against a small rolling y^T buffer,
        # interleaving proj with the next tile's attention on the PE.
        with tc.tile_pool(name="wp", bufs=1) as wpp, \
             tc.tile_pool(name="kv", bufs=2) as kvp, \
             tc.tile_pool(name="qs", bufs=4) as qsp, \
             tc.tile_pool(name="va", bufs=2) as vap, \
             tc.tile_pool(name="e", bufs=3) as ep, \
             tc.tile_pool(name="yr", bufs=3) as yrp, \
             tc.tile_pool(name="sps", bufs=3, space="PSUM") as sps, \
             tc.tile_pool(name="pvps", bufs=2, space="PSUM") as pvps, \
             tc.tile_pool(name="tps", bufs=1, space="PSUM") as tps, \
             tc.tile_pool(name="ops", bufs=2, space="PSUM") as ops, \
             tc.tile_pool(name="osb", bufs=6) as osbp, \
             tc.tile_pool(name="ytmp", bufs=4) as ytmp:
            wp_sb = wpp.tile([128, PCH, C], F32R)
            for b in range(B):
                bt0 = b * T
                kT = kvp.tile([128, T], F32R, tag="kT")
                vT = kvp.tile([128, T], F32R, tag="vT")
                # chunked so the first score/transpose work starts early
                for c4 in range(0, NKC, 4):
                    cs, ce = c4 * 128, (c4 + 4) * 128
                    nc.sync.dma_start(
                        out=kT[:, cs:ce], in_=qkvT[S - 2, :, bt0 + cs : bt0 + ce]
                    )
                    nc.sync.dma_start(
                        out=vT[:, cs:ce], in_=qkvT[S - 1, :, bt0 + cs : bt0 + ce]
                    )

                # V_aug: [tk 128, chunk, 0:128 v / 128 ones / pad]
                va = vap.tile([128, NKC, VW], F16)
                nc.vector.memset(va[:, :, 128:130], 1.0)
                if VW > 130:
                    nc.vector.memset(va[:, :, 130:VW], 0.0)
                for ck in range(NKC):
                    pt = tps.tile([128, 128], F32R, tag="tp")
                    nc.tensor.transpose(
                        pt, vT[:, ck * 128 : (ck + 1) * 128], iden_sb
                    )
                    nc.vector.tensor_copy(out=va[:, ck, 0:128], in_=pt)

                # descending j: the largest query tiles first, so the PE has
                # deep attention work queued while the wp load is in flight
                for j in range(NJ - 1, -1, -1):
                    q0 = j * TQ
                    ncks = TQC * (j + 1)  # key chunks 0 .. TQC*(j+1)-1
                    yroll = yrp.tile([128, QH, TQ], F32R)
                    for h in range(QH):
                        qsl = qsp.tile([128, TQ], F32R)
                        nc.sync.dma_start(
                            out=qsl, in_=qkvT[h, :, bt0 + q0 : bt0 + q0 + TQ]
                        )
                        if b == 0 and j == NJ - 1:
                            # wp is first needed once the first query tile's
                            # attention completes; drip one chunk per head so
                            # the q slices are not stuck behind it in the queue
                            nc.sync.dma_start(out=wp_sb[:, h, :], in_=wpT[h])
                        e_sb = ep.tile([128, NKC, TQ], F16)
                        for ck in range(ncks):
                            ps = sps.tile([128, TQ], F32, tag="s")
                            nc.tensor.matmul(
                                ps,
                                kT[:, ck * 128 : (ck + 1) * 128],
                                qsl,
                                start=True,
                                stop=True,
                            )
                            c = ck - TQC * j
                            lo = 0 if c < 0 else c * 128
                            nc.scalar.activation(
                                out=e_sb[:, ck, lo:TQ],
                                in_=ps[:, lo:TQ],
                                func=mybir.ActivationFunctionType.Exp,
                                scale=SCALE,
                                # keeps E within f16 range; the common factor
                                # cancels exactly in the softmax division
                                bias=ebias_sb,
                            )
                            if c >= 0:
                                nc.vector.tensor_mul(
                                    e_sb[:, ck, lo : lo + 128],
                                    e_sb[:, ck, lo : lo + 128],
                                    tri_sb,
                                )
                        for cq in range(TQC):
                            qc = TQC * j + cq  # global q chunk
                            pv = pvps.tile([128, VW], F32, tag="pv")
                            for ck in range(qc + 1):
                                nc.tensor.matmul(
                                    pv,
                                    e_sb[:, ck, cq * 128 : (cq + 1) * 128],
                                    va[:, ck, :],
                                    start=(ck == 0),
                                    stop=(ck == qc),
                                )
                            rcp = ytmp.tile([128, 1], F32, tag="rcp")
                            nc.vector.reciprocal(rcp, pv[:, 128:129])
                            ysb = ytmp.tile([128, 128], F32R, tag="ysb")
                            nc.vector.tensor_scalar_mul(ysb, pv[:, 0:128], rcp)
                            yt_ps = tps.tile([128, 128], F32R, tag="tp")
                            nc.tensor.transpose(yt_ps, ysb, iden_sb)
                            nc.vector.tensor_copy(
                                out=yroll[:, h, cq * 128 : (cq + 1) * 128],
                                in_=yt_ps,
                            )
                    # output projection for this (b, j) query tile
                    for cq in range(TQC):
                        trow = bt0 + q0 + cq * 128
                        for ot in range(C // 512):
                            po = ops.tile([128, 512], F32)
                            for h in range(PCH):
                                nc.tensor.matmul(
                                    po,
                                    yroll[:, h, cq * 128 : (cq + 1) * 128],
                                    wp_sb[:, h, ot * 512 : (ot + 1) * 512],
                                    start=(h == 0),
                                    stop=(h == PCH - 1),
                                )
                            o_sb = osbp.tile([128, 512], F16)
                            nc.vector.tensor_copy(out=o_sb, in_=po)
                            nc.gpsimd.dma_start(
                                out=out_p[
                                    trow : trow + 128, ot * 512 : (ot + 1) * 512
                                ],
                                in_=o_sb,
                            )

    _split_multi_waits(nc)
    return nc


_PROGRAM = None


def _get_program():
    global _PROGRAM
    if _PROGRAM is None:
        _PROGRAM = _build_program()
    return _PROGRAM


def _prep_inputs(x, cos, sin, W_attn, W_proj):
    x = np.ascontiguousarray(np.asarray(x, dtype=np.float32))
    cos = np.asarray(cos, dtype=np.float32)
    sin = np.asarray(sin, dtype=np.float32)
    W_attn = np.asarray(W_attn, dtype=np.float32)
    W_proj = np.asarray(W_proj, dtype=np.float32)

    xT = np.ascontiguousarray(
        x.reshape(NT, C).T.reshape(KC, 128, NT)
    )
    cosT = cos.T  # (64, T)
    sinT = sin.T
    cc = np.ascontiguousarray(np.concatenate([cosT, cosT], axis=0))
    ss = np.ascontiguousarray(np.concatenate([sinT, -sinT], axis=0))
    ii, jj = np.meshgrid(np.arange(128), np.arange(128), indexing="ij")
    tri = (ii <= jj).astype(np.float16)
    iden = np.eye(128, dtype=np.float32)

    in_maps = []
    for g in range(N_CORES):
        waT = np.ascontiguousarray(
            W_attn[g * GO : (g + 1) * GO, :].T.reshape(KC, 128, GO)
        )
        wpT = np.ascontiguousarray(
            W_proj[:, g * 512 : (g + 1) * 512].T.reshape(PCH, 128, C)
        )
        in_maps.append(
            {
                "xT": xT,
                "waT": waT,
                "wpT": wpT,
                "cc": cc,
                "ss": ss,
                "tri": tri,
                "iden": iden,
            }
        )
    return in_maps


def kernel(x, cos, sin, W_attn, W_proj, _return_results=False, **trace_kwargs):
    nc = _get_program()
    in_maps = _prep_inputs(x, cos, sin, W_attn, W_proj)
    res = run_bass_kernel_spmd(nc, in_maps, list(range(N_CORES)), **trace_kwargs)
    acc = res.results[0]["out_p"].astype(np.float32)
    for g in range(1, N_CORES):
        acc = acc + res.results[g]["out_p"].astype(np.float32)
    out = acc.reshape(B, T, C)
    if _return_results:
        return out, res
    return out


# revision 43
# speedup vs baseline: 1.0917x; 1.0365x over previous
"""Causal self-attention (GQA) Trainium2 Bass kernel.

Config (hardcoded): B=2, T=2048, C=4096, n_head=32, n_query_groups=8,
head_size=128, qkv_out=6144. RoPE (NeoX, rotary dim = head_size).

Sharding: tensor-parallel over the 8 KV groups, one group per NeuronCore.
Each core computes its group's QKV projection, RoPE, causal attention for
its 4 query heads, and a partial output projection against its 512 columns
of W_proj. The 8 fp32 partials are summed on the host (the all-reduce is
host-side since full outputs are gathered anyway).

Everything on the PE runs in float32r (fp32 storage, PE rounds to its
fast-fp32 format): ~1.5e-4 matmul rel err vs 2.3e-3 for bf16, at full PE
rate when the moving dim is >=256.

Attention is computed in the S^T = K@Q^T orientation so the softmax
reduction lands on the free axis sum via a ones-column in the PV matmul,
and no P-transposes are needed (only V and the final per-head Y tiles get
PE transposes).
"""

import math
from contextlib import ExitStack

import numpy as np

import concourse.bass as bass
import concourse.tile as tile
from concourse import mybir
from concourse.bass_utils import run_bass_kernel_spmd
from concourse.tile import TileContext
from concourse.vector_clock import ScopedClock

# ---------------------------------------------------------------------------
# Workarounds: walrus codegen on this toolchain rejects >1 sync wait per
# instruction ("Too many sync wait commands"). Split excess waits onto
# extra SP drains (TileContext tail) / same-engine InstNoOp instructions.
# ---------------------------------------------------------------------------

_MAX_WAITS = 1


def _patched_drain_and_barrier(self, tick_clock, wait_clock):
    drain_inst = self.nc.sync.drain()
    wait_clock.add_sem_waits(
        drain_inst.ins, ScopedClock({None: tick_clock.global_clock})
    )
    si = drain_inst.ins.sync_info
    waits = list(si.on_wait or [])
    if len(waits) > _MAX_WAITS:
        si.on_wait = waits[:_MAX_WAITS]
        rest = waits[_MAX_WAITS:]
        for i in range(0, len(rest), _MAX_WAITS):
            extra = self.nc.sync.drain()
            extra.ins.sync_info = type(si)(
                on_wait=rest[i : i + _MAX_WAITS], on_update=[]
            )

    self.nc.all_engine_barrier()
    assert self.sems is not None
    popped = self.nc._tile_sem_poison_stack.pop()
    assert popped is self._sem_poison
    self.nc.clear_and_free_semaphores(list(self.sems.allocated().values()))
    self.nc.all_engine_barrier()


tile.TileContext._drain_and_barrier = _patched_drain_and_barrier

_split_counter = [0]


def _split_multi_waits(nc, max_waits=_MAX_WAITS):
    for fn in nc.m.functions:
        for bb in fn.blocks:
            out = []
            for inst in bb.instructions:
                si = inst.sync_info
                waits = list(si.on_wait) if si and si.on_wait else []
                if len(waits) > max_waits:
                    rest = waits[: len(waits) - max_waits]
                    si.on_wait = waits[len(waits) - max_waits :]
                    for i in range(0, len(rest), max_waits):
                        _split_counter[0] += 1
                        nop = mybir.InstNoOp(
                            name=f"I-wsplit-{_split_counter[0]}", ins=[], outs=[]
                        )
                        nop.engine = inst.engine
                        nop.sync_info = type(si)(
                            on_wait=rest[i : i + max_waits], on_update=[]
                        )
                        out.append(nop)
                out.append(inst)
            bb.instructions = out


# ---------------------------------------------------------------------------
# Problem constants
# ---------------------------------------------------------------------------

B, T, C = 2, 2048, 4096
G, QH, D = 8, 4, 128          # kv groups, q heads per group, head dim
S = QH + 2                    # s-blocks per group: q0..q3, k, v
GO = S * D                    # 768 rows of W_attn per group
NT = B * T                    # 4096 tokens
KC = C // 128                 # 32 contraction chunks over C
TT = 256                      # token tile for the QKV matmul
NTT = NT // TT                # 16 token tiles
TQ = 256                      # query tile in attention
TQC = TQ // 128               # q chunks per tile
NJ = T // TQ                  # query tiles per batch
NKC = T // 128                # 16 key chunks per batch
PCH = 4                       # proj contraction chunks (512 = 4*128)
VW = 132                      # PV rhs width (129 used; f16 runs 1cyc/row at any width)
SCALE = 1.0 / math.sqrt(D)
EXP_BIAS = -9.0

F32R = mybir.dt.float32r
F32 = mybir.dt.float32
F16 = mybir.dt.float16

N_CORES = 8


def _build_program():
    nc = bass.Bass("TRN2", target_bir_lowering=False, debug=False,
                   num_devices=N_CORES)

    xT = nc.dram_tensor("xT", [KC, 128, NT], F32R, kind="ExternalInput").ap()
    waT = nc.dram_tensor("waT", [KC, 128, GO], F32R, kind="ExternalInput").ap()
    wpT = nc.dram_tensor("wpT", [PCH, 128, C], F32R, kind="ExternalInput").ap()
    cc = nc.dram_tensor("cc", [128, T], F32, kind="ExternalInput").ap()
    ss = nc.dram_tensor("ss", [128, T], F32, kind="ExternalInput").ap()
    tri = nc.dram_tensor("tri", [128, 128], F16, kind="ExternalInput").ap()
    iden = nc.dram_tensor("iden", [128, 128], F32R, kind="ExternalInput").ap()
    out_p = nc.dram_tensor("out_p", [NT, C], F16, kind="ExternalOutput").ap()

    with TileContext(nc) as tc, ExitStack() as ctx:
        consts = ctx.enter_context(tc.tile_pool(name="consts", bufs=1))
        dram = ctx.enter_context(tc.tile_pool(name="dram", bufs=1, space="DRAM"))

        tri_sb = consts.tile([128, 128], F16)
        iden_sb = consts.tile([128, 128], F32R)
        ebias_sb = consts.tile([128, 1], F32)
        nc.vector.memset(ebias_sb, EXP_BIAS)

        # qkv^T scratch in DRAM: [s-block][d][token]
        qkvT = dram.tile([S, 128, NT], F32R)

        # ------------------- Phase 1: QKV projection + RoPE -------------------
        with tc.tile_pool(name="wa", bufs=1) as wap, \
             tc.tile_pool(name="xs", bufs=2) as xsp, \
             tc.tile_pool(name="ropec", bufs=1) as ropec, \
             tc.tile_pool(name="qkps", bufs=4, space="PSUM") as qkps, \
             tc.tile_pool(name="rope", bufs=2) as ropep:
            cc_sb = ropec.tile([128, T], F32)
            ss_sb = ropec.tile([128, T], F32)
            wa_sb = wap.tile([128, KC, GO], F32R)
            # k/v s-blocks first: attention's kT/vT loads depend on the last
            # token-tile's k/v writes, so emit those as early as possible
            M_ORDER = [S - 2, S - 1, 0, 1, 2, 3]

            def _wa_chunk(m, klo=0, khi=KC):
                nc.sync.dma_start(
                    out=wa_sb[:, klo:khi, m * 128 : (m + 1) * 128],
                    in_=waT[klo:khi, :, m * 128 : (m + 1) * 128].rearrange(
                        "k p o -> p k o"
                    ),
                )

            x_tiles = {}

            def _load_x(tt, chunked=False):
                t0 = tt * TT
                xt = xsp.tile([128, KC, TT], F32R, tag="x")
                if chunked:
                    for klo in range(0, KC, 8):
                        nc.sync.dma_start(
                            out=xt[:, klo : klo + 8, :],
                            in_=xT[klo : klo + 8, :, t0 : t0 + TT].rearrange(
                                "k p t -> p k t"
                            ),
                        )
                else:
                    nc.sync.dma_start(
                        out=xt,
                        in_=xT[:, :, t0 : t0 + TT].rearrange("k p t -> p k t"),
                    )
                x_tiles[tt] = xt

            # fine-grained startup: interleave first weight chunk octets with
            # the first x tile octets so matmuls start within a few us and
            # stay fed at the DMA drip rate
            m0 = M_ORDER[0]
            xt0 = xsp.tile([128, KC, TT], F32R, tag="x")
            for klo in range(0, KC, 8):
                _wa_chunk(m0, klo, klo + 8)
                nc.sync.dma_start(
                    out=xt0[:, klo : klo + 8, :],
                    in_=xT[klo : klo + 8, :, 0:TT].rearrange("k p t -> p k t"),
                )
            x_tiles[0] = xt0

            for tt in range(NTT):
                t0 = tt * TT
                x_sb = x_tiles.pop(tt)
                # within-batch column range for the rope tables
                r0 = t0 % T
                for mi, m in enumerate(M_ORDER):
                    ps = qkps.tile([128, TT], F32)
                    for k in range(KC):
                        nc.tensor.matmul(
                            ps,
                            wa_sb[:, k, m * 128 : (m + 1) * 128],
                            x_sb[:, k, :],
                            start=(k == 0),
                            stop=(k == KC - 1),
                        )
                    if mi == 0:
                        if tt == 0:
                            # deferred loads interleaved in PE need-order
                            xt1 = xsp.tile([128, KC, TT], F32R, tag="x")

                            def _x1_oct(klo):
                                nc.sync.dma_start(
                                    out=xt1[:, klo : klo + 8, :],
                                    in_=xT[klo : klo + 8, :, TT : 2 * TT].rearrange(
                                        "k p t -> p k t"
                                    ),
                                )

                            _wa_chunk(M_ORDER[1])
                            _x1_oct(0)
                            _wa_chunk(M_ORDER[2])
                            _x1_oct(8)
                            nc.sync.dma_start(out=cc_sb, in_=cc)
                            nc.sync.dma_start(out=ss_sb, in_=ss)
                            _wa_chunk(M_ORDER[3])
                            _x1_oct(16)
                            _wa_chunk(M_ORDER[4])
                            _x1_oct(24)
                            _wa_chunk(M_ORDER[5])
                            nc.sync.dma_start(out=tri_sb, in_=tri)
                            nc.sync.dma_start(out=iden_sb, in_=iden)
                            x_tiles[1] = xt1
                        elif tt + 1 < NTT:
                            _load_x(tt + 1)
                    if m != S - 1:
                        # RoPE directly from PSUM with partition-offset reads:
                        #   out = ps*CC - rot(ps)*SS, rot = half-swap of ps
                        t1 = ropep.tile([128, TT], F32, tag="t1")
                        nc.vector.tensor_mul(t1, ps, cc_sb[:, r0 : r0 + TT])
                        t2 = ropep.tile([128, TT], F32, tag="t2")
                        nc.vector.tensor_mul(
                            t2[0:64, :], ps[64:128, :], ss_sb[0:64, r0 : r0 + TT]
                        )
                        nc.vector.tensor_mul(
                            t2[64:128, :], ps[0:64, :], ss_sb[64:128, r0 : r0 + TT]
                        )
                        qr = ropep.tile([128, TT], F32R, tag="qr")
                        nc.vector.tensor_sub(qr, t1, t2)
                        nc.gpsimd.dma_start(out=qkvT[m, :, t0 : t0 + TT], in_=qr)
                    else:
                        vr = ropep.tile([128, TT], F32R, tag="qr")
                        nc.scalar.copy(out=vr, in_=ps)
                        nc.gpsimd.dma_start(out=qkvT[m, :, t0 : t0 + TT], in_=vr)

        # --------- Phase 2: fused attention + output projection ---------
        # j-outer / h-inner; after all 4 heads finish a query tile, its
        # output-projection matmuls run against a small rolling y^T buffer,
        # interleaving proj with the next tile's attention on the PE.
        with tc.tile_pool(name="wp", bufs=1) as wpp, \
             tc.tile_pool(name="kv", bufs=2) as kvp, \
             tc.tile_pool(name="qs", bufs=4) as qsp, \
             tc.tile_pool(name="va", bufs=2) as vap, \
             tc.tile_pool(name="e", bufs=3) as ep, \
             tc.tile_pool(name="yr", bufs=3) as yrp, \
             tc.tile_pool(name="sps", bufs=3, space="PSUM") as sps, \
             tc.tile_pool(name="pvps", bufs=2, space="PSUM") as pvps, \
             tc.tile_pool(name="tps", bufs=1, space="PSUM") as tps, \
             tc.tile_pool(name="ops", bufs=2, space="PSUM") as ops, \
             tc.tile_pool(name="osb", bufs=6) as osbp, \
             tc.tile_pool(name="ytmp", bufs=4) as ytmp:
            wp_sb = wpp.tile([128, PCH, C], F32R)
            for b in range(B):
                bt0 = b * T
                kT = kvp.tile([128, T], F32R, tag="kT")
                vT = kvp.tile([128, T], F32R, tag="vT")
                # chunked so the first score/transpose work starts early
                for c4 in range(0, NKC, 4):
                    cs, ce = c4 * 128, (c4 + 4) * 128
                    nc.sync.dma_start(
                        out=kT[:, cs:ce], in_=qkvT[S - 2, :, bt0 + cs : bt0 + ce]
                    )
                    nc.sync.dma_start(
                        out=vT[:, cs:ce], in_=qkvT[S - 1, :, bt0 + cs : bt0 + ce]
                    )

                # V_aug: [tk 128, chunk, 0:128 v / 128 ones / pad]
                va = vap.tile([128, NKC, VW], F16)
                nc.vector.memset(va[:, :, 128:130], 1.0)
                if VW > 130:
                    nc.vector.memset(va[:, :, 130:VW], 0.0)
                for ck in range(NKC):
                    pt = tps.tile([128, 128], F32R, tag="tp")
                    nc.tensor.transpose(
                        pt, vT[:, ck * 128 : (ck + 1) * 128], iden_sb
                    )
                    nc.vector.tensor_copy(out=va[:, ck, 0:128], in_=pt)

                # descending j: the largest query tiles first, so the PE has
                # deep attention work queued while the wp load is in flight
                for j in range(NJ - 1, -1, -1):
                    q0 = j * TQ
                    ncks = TQC * (j + 1)  # key chunks 0 .. TQC*(j+1)-1
                    yroll = yrp.tile([128, QH, TQ], F32R)
                    for h in range(QH):
                        qsl = qsp.tile([128, TQ], F32R)
                        nc.sync.dma_start(
                            out=qsl, in_=qkvT[h, :, bt0 + q0 : bt0 + q0 + TQ]
                        )
                        if b == 0 and j == NJ - 1:
                            # wp is first needed once the first query tile's
                            # attention completes; drip one chunk per head so
                            # the q slices are not stuck behind it in the queue
                            nc.sync.dma_start(out=wp_sb[:, h, :], in_=wpT[h])
                        e_sb = ep.tile([128, NKC, TQ], F16)
                        # key chunks in pairs per PSUM tile: one exp op per
                        # non-diagonal pair (fewer ACT ops and handoffs)
                        for ckp in range(0, ncks, 2):
                            ps = sps.tile([128, 2, TQ], F32, tag="s")
                            for ci in range(2):
                                nc.tensor.matmul(
                                    ps[:, ci, :],
                                    kT[:, (ckp + ci) * 128 : (ckp + ci + 1) * 128],
                                    qsl,
                                    start=True,
                                    stop=True,
                                )
                            if ckp < TQC * j:
                                nc.scalar.activation(
                                    out=e_sb[:, ckp : ckp + 2, :],
                                    in_=ps,
                                    func=mybir.ActivationFunctionType.Exp,
                                    scale=SCALE,
                                    # keeps E within f16 range; the common
                                    # factor cancels in the softmax division
                                    bias=ebias_sb,
                                )
                            else:
                                # diagonal pair: chunk ckp masks tq [0:128],
                                # chunk ckp+1 only covers tq [128:TQ]
                                nc.scalar.activation(
                                    out=e_sb[:, ckp, :],
                                    in_=ps[:, 0, :],
                                    func=mybir.ActivationFunctionType.Exp,
                                    scale=SCALE,
                                    bias=ebias_sb,
                                )
                                nc.vector.tensor_mul(
                                    e_sb[:, ckp, 0:128],
                                    e_sb[:, ckp, 0:128],
                                    tri_sb,
                                )
                                nc.scalar.activation(
                                    out=e_sb[:, ckp + 1, 128:TQ],
                                    in_=ps[:, 1, 128:TQ],
                                    func=mybir.ActivationFunctionType.Exp,
                                    scale=SCALE,
                                    bias=ebias_sb,
                                )
                                nc.vector.tensor_mul(
                                    e_sb[:, ckp + 1, 128:TQ],
                                    e_sb[:, ckp + 1, 128:TQ],
                                    tri_sb,
                                )
                        for cq in range(TQC):
                            qc = TQC * j + cq  # global q chunk
                            pv = pvps.tile([128, VW], F32, tag="pv")
                            for ck in range(qc + 1):
                                nc.tensor.matmul(
                                    pv,
                                    e_sb[:, ck, cq * 128 : (cq + 1) * 128],
                                    va[:, ck, :],
                                    start=(ck == 0),
                                    stop=(ck == qc),
                                )
                            rcp = ytmp.tile([128, 1], F32, tag="rcp")
                            nc.vector.reciprocal(rcp, pv[:, 128:129])
                            ysb = ytmp.tile([128, 128], F32R, tag="ysb")
                            nc.vector.tensor_scalar_mul(ysb, pv[:, 0:128], rcp)
                            yt_ps = tps.tile([128, 128], F32R, tag="tp")
                            nc.tensor.transpose(yt_ps, ysb, iden_sb)
                            nc.vector.tensor_copy(
                                out=yroll[:, h, cq * 128 : (cq + 1) * 128],
                                in_=yt_ps,
                            )
                    # output projection for this (b, j) query tile
                    for cq in range(TQC):
                        trow = bt0 + q0 + cq * 128
                        for ot in range(C // 512):
                            po = ops.tile([128, 512], F32)
                            for h in range(PCH):
                                nc.tensor.matmul(
                                    po,
                                    yroll[:, h, cq * 128 : (cq + 1) * 128],
                                    wp_sb[:, h, ot * 512 : (ot + 1) * 512],
                                    start=(h == 0),
                                    stop=(h == PCH - 1),
                                )
                            o_sb = osbp.tile([128, 512], F16)
                            nc.vector.tensor_copy(out=o_sb, in_=po)
                            nc.gpsimd.dma_start(
                                out=out_p[
                                    trow : trow + 128, ot * 512 : (ot + 1) * 512
                                ],
                                in_=o_sb,
                            )

    _split_multi_waits(nc)
    return nc


_PROGRAM = None


def _get_program():
    global _PROGRAM
    if _PROGRAM is None:
        _PROGRAM = _build_program()
    return _PROGRAM


def _prep_inputs(x, cos, sin, W_attn, W_proj):
    x = np.ascontiguousarray(np.asarray(x, dtype=np.float32))
    cos = np.asarray(cos, dtype=np.float32)
    sin = np.asarray(sin, dtype=np.float32)
    W_attn = np.asarray(W_attn, dtype=np.float32)
    W_proj = np.asarray(W_proj, dtype=np.float32)

    xT = np.ascontiguousarray(
        x.reshape(NT, C).T.reshape(KC, 128, NT)
    )
    cosT = cos.T  # (64, T)
    sinT = sin.T
    cc = np.ascontiguousarray(np.concatenate([cosT, cosT], axis=0))
    ss = np.ascontiguousarray(np.concatenate([sinT, -sinT], axis=0))
    ii, jj = np.meshgrid(np.arange(128), np.arange(128), indexing="ij")
    tri = (ii <= jj).astype(np.float16)
    iden = np.eye(128, dtype=np.float32)

    in_maps = []
    for g in range(N_CORES):
        waT = np.ascontiguousarray(
            W_attn[g * GO : (g + 1) * GO, :].T.reshape(KC, 128, GO)
        )
        wpT = np.ascontiguousarray(
            W_proj[:, g * 512 : (g + 1) * 512].T.reshape(PCH, 128, C)
        )
        in_maps.append(
            {
                "xT": xT,
                "waT": waT,
                "wpT": wpT,
                "cc": cc,
                "ss": ss,
                "tri": tri,
                "iden": iden,
            }
        )
    return in_maps


def kernel(x, cos, sin, W_attn, W_proj, _return_results=False, **trace_kwargs):
    nc = _get_program()
    in_maps = _prep_inputs(x, cos, sin, W_attn, W_proj)
    res = run_bass_kernel_spmd(nc, in_maps, list(range(N_CORES)), **trace_kwargs)
    acc = res.results[0]["out_p"].astype(np.float32)
    for g in range(1, N_CORES):
        acc = acc + res.results[g]["out_p"].astype(np.float32)
    out = acc.reshape(B, T, C)
    if _return_results:
        return out, res
    return out
